# revision 1
# baseline (speedup 1.0000x reference)
"""CoAtNet transformer block kernel for Trainium2 (8 NeuronCores).

Strategy:
  - Data-parallel over batch: 64 images -> 8 per core, no collectives.
  - Channel-major activation layout [C, N] on chip (x arrives as (C, H*W)).
  - All matmuls in bf16 (fp32 PSUM accumulation); LN stats / residuals fp32.
  - LayerNorm gamma/beta folded into QKV weights host-side. Q/K projections
    run on raw (un-normalized) bf16 x so they never wait on the LN stats
    chain; the per-token (mean, rstd) correction is applied to the PSUM
    result as rstd*psum + mr*colsum(w) (+bias) on the vector/scalar engines.
  - Attention computed transposed (simT[m, n] = k@q.T + biasT) so softmax
    normalization is a column sum obtained for free from a ones-column in
    the V matmul; bias applied as precomputed exp(biasT) multiplier.
  - Batches processed in pairs so the moving free dim is 392 (hides
    LDWEIGHTS under the matmul stream); per-head sim matmuls packed two
    heads at a time into disjoint PE row groups.
  - FFN: per 128-wide h1 chunk, gelu then immediately accumulate into six
    persistent output PSUM banks (no full h1 materialization).
"""

import numpy as np
import ml_dtypes

H = 14
W = 14
C = 768
HEADS = 12
EXPAND = 4
N = H * W  # 196
B = 64
NCORES = 8
BPC = B // NCORES  # 8 batches per core
DH = C // HEADS  # 64
KC = C // 128  # 6 chunks of 128 channels
F = C * EXPAND  # 3072
KF = F // 128  # 24
NPAIR = 2 * N  # 392
M0, M1 = 128, N - 128  # token chunks 128 + 68
MCHUNKS = ((0, M0), (M0, M1))


def _relative_indices():
    gy, gx = np.meshgrid(np.arange(H), np.arange(W), indexing="ij")
    py, px = gy.reshape(-1), gx.reshape(-1)
    rel_y = py[None, :] - py[:, None] + H
    rel_x = px[None, :] - px[:, None] + W
    return rel_y * W + rel_x  # (N, N) int


def _build_bass():
    import concourse.bacc as bacc
    import concourse.mybir as mybir
    import concourse.tile as tile

    f32 = mybir.dt.float32
    bf16 = mybir.dt.bfloat16
    AF = mybir.ActivationFunctionType
    OP = mybir.AluOpType

    nc = bacc.Bacc("TRN2")

    # ---- DRAM parameters (per core) ----
    x_in = nc.declare_dram_parameter("x", [BPC, C, N], f32, isOutput=False)
    wq_d = nc.declare_dram_parameter("wq", [C, C], bf16, isOutput=False)
    wk_d = nc.declare_dram_parameter("wk", [C, C], bf16, isOutput=False)
    wv_d = nc.declare_dram_parameter("wv", [C, C], bf16, isOutput=False)
    wo_d = nc.declare_dram_parameter("wo", [C, C], bf16, isOutput=False)
    w1_d = nc.declare_dram_parameter("w1", [C, F], bf16, isOutput=False)
    w2_d = nc.declare_dram_parameter("w2", [F, C], bf16, isOutput=False)
    bq_d = nc.declare_dram_parameter("bq", [C], f32, isOutput=False)
    bk_d = nc.declare_dram_parameter("bk", [C], f32, isOutput=False)
    bo_d = nc.declare_dram_parameter("bo", [C], f32, isOutput=False)
    b1_d = nc.declare_dram_parameter("b1", [F], f32, isOutput=False)
    b2_d = nc.declare_dram_parameter("b2", [C], f32, isOutput=False)
    wsq_d = nc.declare_dram_parameter("wsq", [C], f32, isOutput=False)
    wsk_d = nc.declare_dram_parameter("wsk", [C], f32, isOutput=False)
    # exp(biasT) per head, token-chunked: [128, HEADS, N] and [68, HEADS, N]
    eb0_d = nc.declare_dram_parameter("eb0", [M0, HEADS, N], bf16, isOutput=False)
    eb1_d = nc.declare_dram_parameter("eb1", [M1, HEADS, N], bf16, isOutput=False)
    out_d = nc.declare_dram_parameter("out", [BPC, C, N], f32, isOutput=True)

    def ld(pool, name, dram, shape, pat):
        t = pool.tile(shape, dram.dtype, name=name)
        nc.sync.dma_start(t[:], dram.ap().rearrange(pat, p=128) if pat else dram.ap())
        return t

    with tile.TileContext(nc) as tc:
        with (
            tc.tile_pool(name="wpool", bufs=1) as wpool,
            tc.tile_pool(name="acts", bufs=1) as acts,
            tc.tile_pool(name="xio", bufs=2) as xio,
            tc.tile_pool(name="small", bufs=2) as small,
            tc.tile_pool(name="psum", bufs=1, space="PSUM") as pp,
        ):
            # ---- resident weights; issue order = DMA priority order ----
            wq_sb = ld(wpool, "wq_sb", wq_d, [128, KC, C], "(ko p) m -> p ko m")
            wk_sb = ld(wpool, "wk_sb", wk_d, [128, KC, C], "(ko p) m -> p ko m")
            bq_sb = ld(wpool, "bq_sb", bq_d, [128, KC], "(mo p) -> p mo")
            bk_sb = ld(wpool, "bk_sb", bk_d, [128, KC], "(mo p) -> p mo")
            wsq_sb = ld(wpool, "wsq_sb", wsq_d, [128, KC], "(mo p) -> p mo")
            wsk_sb = ld(wpool, "wsk_sb", wsk_d, [128, KC], "(mo p) -> p mo")
            wv_sb = ld(wpool, "wv_sb", wv_d, [128, KC, C], "(ko p) m -> p ko m")
            eb0_sb = ld(wpool, "eb0_sb", eb0_d, [M0, HEADS, N], None)
            eb1_sb = ld(wpool, "eb1_sb", eb1_d, [M1, HEADS, N], None)
            bo_sb = ld(wpool, "bo_sb", bo_d, [128, KC], "(mo p) -> p mo")
            b1_sb = ld(wpool, "b1_sb", b1_d, [128, KF], "(mo p) -> p mo")
            b2_sb = ld(wpool, "b2_sb", b2_d, [128, KC], "(mo p) -> p mo")
            wo_sb = ld(wpool, "wo_sb", wo_d, [128, KC, C], "(ko p) m -> p ko m")
            w1_sb = ld(wpool, "w1_sb", w1_d, [128, KC, F], "(ko p) m -> p ko m")
            w2_sb = ld(wpool, "w2_sb", w2_d, [128, KF, C], "(ko p) m -> p ko m")
            ones_sb = wpool.tile([128, 1], bf16, name="ones_sb")
            nc.vector.memset(ones_sb[:], 1.0)
            eps_sb = wpool.tile([1, 1], f32, name="eps_sb")
            nc.vector.memset(eps_sb[:], 1e-5)
            zero_sb = wpool.tile([128, 1], f32, name="zero_sb")
            nc.vector.memset(zero_sb[:], 0.0)

            eb_sb = (eb0_sb, eb1_sb)

            def load_x(pair):
                b0 = 2 * pair
                t = xio.tile([128, 2 * KC, N], f32, name="xf", tag="xf")
                nc.sync.dma_start(
                    t[:],
                    x_in.ap()[b0 : b0 + 2].rearrange(
                        "b (ko p) n -> p (b ko) n", p=128
                    ),
                )
                return t

            next_xf = load_x(0)
            for pair in range(BPC // 2):
                b0 = 2 * pair
                xf = next_xf
                if pair + 1 < BPC // 2:
                    next_xf = load_x(pair + 1)
                # pair view: [128, KC, 2, N] (chunk-major, batch inner)
                xfv = xf.rearrange("p (b k) n -> p k b n", b=2)

                # ---- bf16 cast of x (DVE) + LN stats matmuls ----
                xbf = acts.tile([128, KC, 2, N], bf16, name="xbf", tag="xbf")
                s_ps = pp.tile([1, NPAIR], f32, name="s_ps", tag="x0")
                q_ps = pp.tile([1, NPAIR], f32, name="q_ps", tag="x1")
                for k in range(KC):
                    nc.vector.tensor_copy(xbf[:, k], xfv[:, k])
                    xsq = small.tile([128, 2, N], bf16, name="xsq", tag="xsq")
                    nc.gpsimd.tensor_mul(xsq[:], xbf[:, k], xbf[:, k])
                    nc.tensor.matmul(
                        s_ps[:], ones_sb[:], xbf[:, k],
                        start=(k == 0), stop=(k == KC - 1),
                    )
                    nc.tensor.matmul(
                        q_ps[:], ones_sb[:], xsq[:],
                        start=(k == 0), stop=(k == KC - 1),
                    )

                # ---- LN stats chain (off PE critical path) ----
                mu = small.tile([1, NPAIR], f32, name="mu", tag="mu", bufs=1)
                nc.vector.tensor_scalar_mul(mu[:], s_ps[:], 1.0 / C)
                var = small.tile([1, NPAIR], f32, name="var", tag="var", bufs=1)
                # var = (mu * -mu) + sq/C
                nc.vector.scalar_tensor_tensor(
                    var[:], mu[:], -1.0, mu[:], OP.mult, OP.mult
                )
                nc.vector.scalar_tensor_tensor(
                    var[:], q_ps[:], 1.0 / C, var[:], OP.mult, OP.add
                )
                std = small.tile([1, NPAIR], f32, name="std", tag="std", bufs=1)
                nc.scalar.activation(
                    std[:], var[:], AF.Sqrt, bias=eps_sb[:], scale=1.0
                )
                rstd = small.tile([1, NPAIR], f32, name="rstd", tag="rstd", bufs=1)
                nc.vector.reciprocal(rstd[:], std[:])
                mr = small.tile([1, NPAIR], f32, name="mr", tag="mr", bufs=1)
                nc.vector.scalar_tensor_tensor(
                    mr[:], mu[:], -1.0, rstd[:], OP.mult, OP.mult
                )
                rstd_b = small.tile(
                    [128, NPAIR], f32, name="rstd_b", tag="rstd_b", bufs=1
                )
                nc.gpsimd.partition_broadcast(rstd_b[:], rstd[:])
                mr_b = small.tile([128, NPAIR], f32, name="mr_b", tag="mr_b", bufs=1)
                nc.gpsimd.partition_broadcast(mr_b[:], mr[:])
                rstd_bv = rstd_b.rearrange("p (b n) -> p b n", b=2)
                mr_bv = mr_b.rearrange("p (b n) -> p b n", b=2)

                # ---- Q/K projections from RAW x_bf; LN applied post-hoc:
                #      qT = rstd*(w'.T@x) + mr*colsum(w') + b' ----
                qT = acts.tile([128, KC, NPAIR], bf16, name="qT", tag="qT")
                kT = acts.tile([128, KC, NPAIR], bf16, name="kT", tag="kT")
                for dst, w_sb, ws_sb, bias_sb in (
                    (qT, wq_sb, wsq_sb, bq_sb),
                    (kT, wk_sb, wsk_sb, bk_sb),
                ):
                    for m in range(KC):
                        ps = pp.tile(
                            [128, NPAIR], f32, name="ps_qk", tag="mm", bufs=2
                        )
                        for k in range(KC):
                            nc.tensor.matmul(
                                ps[:],
                                w_sb[:, k, 128 * m : 128 * (m + 1)],
                                xbf[:, k],
                                start=(k == 0),
                                stop=(k == KC - 1),
                            )
                        t1 = small.tile([128, NPAIR], f32, name="t1", tag="t1")
                        nc.vector.tensor_mul(t1[:], ps[:], rstd_b[:])
                        t2 = small.tile([128, NPAIR], bf16, name="t2", tag="t2")
                        nc.vector.scalar_tensor_tensor(
                            t2[:], mr_b[:], ws_sb[:, m : m + 1], t1[:],
                            OP.mult, OP.add,
                        )
                        nc.scalar.activation(
                            dst[:, m, :], t2[:], AF.Identity,
                            bias=bias_sb[:, m : m + 1], scale=1.0,
                        )

                # ---- xn = x*rstd + mr (bf16; only feeds the V projection) ----
                xn = acts.tile([128, KC, NPAIR], bf16, name="xn", tag="xn")
                xnv = xn.rearrange("p k (b n) -> p k b n", b=2)
                for k in range(KC):
                    t = small.tile([128, NPAIR], f32, name="t_ln", tag="t1")
                    nc.vector.tensor_mul(
                        t.rearrange("p (b n) -> p b n", b=2), xfv[:, k], rstd_bv
                    )
                    nc.vector.tensor_add(
                        xnv[:, k], t.rearrange("p (b n) -> p b n", b=2), mr_bv
                    )

                # ---- V projection (token-major, per batch, 65-strided heads) ----
                v_sb = []
                for j in range(2):
                    v_c = []
                    for ci, (mstart, mlen) in enumerate(MCHUNKS):
                        vt = acts.tile(
                            [128, HEADS, DH + 1], bf16, name=f"v_{j}_{ci}",
                            tag=f"v_{j}_{ci}",
                        )
                        for s in range(2):  # halves of c_out (6 heads each)
                            pv = pp.tile(
                                [128, 384], f32, name="pv", tag="mm", bufs=2
                            )
                            for k in range(KC):
                                nc.tensor.matmul(
                                    pv[:mlen],
                                    xn[:, k, j * N + mstart : j * N + mstart + mlen],
                                    wv_sb[:, k, 384 * s : 384 * (s + 1)],
                                    start=(k == 0),
                                    stop=(k == KC - 1),
                                )
                            nc.scalar.activation(
                                vt[:mlen, 6 * s : 6 * (s + 1), 0:DH],
                                pv[:mlen].rearrange("p (h d) -> p h d", h=6),
                                AF.Identity, bias=zero_sb[:mlen], scale=1.0,
                            )
                        nc.vector.memset(vt[:mlen, :, DH : DH + 1], 1.0)
                        v_c.append(vt)
                    v_sb.append(v_c)

                # ---- attention: head-pairs packed into PE row groups;
                #      emission software-pipelined one pair ahead ----
                OT = acts.tile([128, KC, NPAIR], bf16, name="OT", tag="OT")

                def emit_sims(j, hp):
                    cb = j * N
                    sims, ets = [], []
                    for hh in range(2):  # heads 2*hp, 2*hp+1
                        prow = 64 * hh
                        sim = pp.tile(
                            [128, 2, N], f32, name="sim", tag=f"st{hh}"
                        )
                        for ci, (mstart, mlen) in enumerate(MCHUNKS):
                            nc.tensor.matmul(
                                sim[:mlen, ci],
                                kT[prow : prow + 64, hp,
                                   cb + mstart : cb + mstart + mlen],
                                qT[prow : prow + 64, hp, cb : cb + N],
                                start=True, stop=True,
                            )
                        et = small.tile(
                            [128, 2, N], bf16, name="et", tag=f"et{hh}"
                        )
                        nc.scalar.activation(
                            et[:], sim[:], AF.Exp, bias=0.0, scale=1.0
                        )
                        for ci, (mstart, mlen) in enumerate(MCHUNKS):
                            nc.vector.tensor_mul(
                                et[:mlen, ci], et[:mlen, ci],
                                eb_sb[ci][:mlen, 2 * hp + hh, :],
                            )
                        sims.append(sim)
                        ets.append(et)
                    return ets

                def emit_omms(j, hp, ets):
                    cb = j * N
                    for hh in range(2):
                        h = 2 * hp + hh
                        _otags = ["at0", "at1", "x0", "x1"]
                        ops = pp.tile(
                            [128, N], f32, name="ops", tag=_otags[(2 * hp + hh) % 4]
                        )
                        for ci, (mstart, mlen) in enumerate(MCHUNKS):
                            nc.tensor.matmul(
                                ops[: DH + 1],
                                v_sb[j][ci][:mlen, h, :],
                                ets[hh][:mlen, ci],
                                start=(ci == 0), stop=(ci == 1),
                            )
                        rec = small.tile([1, N], f32, name="rec", tag="rec")
                        nc.vector.reciprocal(rec[:], ops[DH : DH + 1, :])
                        rec_b = small.tile([64, N], f32, name="rec_b", tag="rec_b")
                        nc.gpsimd.partition_broadcast(rec_b[:], rec[:])
                        nc.vector.tensor_mul(
                            OT[64 * hh : 64 * hh + 64, hp, cb : cb + N],
                            ops[0:DH, :], rec_b[:],
                        )

                prev = None
                for j in range(2):
                    for hp in range(HEADS // 2):
                        ets = emit_sims(j, hp)
                        if prev is not None:
                            emit_omms(*prev)
                        prev = (j, hp, ets)
                emit_omms(*prev)

                # ---- out projection + residual 1 ----
                y32 = acts.tile([128, KC, NPAIR], f32, name="y32", tag="y32")
                ybf = acts.tile([128, KC, NPAIR], bf16, name="ybf", tag="ybf")
                for m in range(KC):
                    po = pp.tile([128, NPAIR], f32, name="po", tag="mm", bufs=2)
                    for k in range(KC):
                        nc.tensor.matmul(
                            po[:],
                            wo_sb[:, k, 128 * m : 128 * (m + 1)],
                            OT[:, k, :],
                            start=(k == 0),
                            stop=(k == KC - 1),
                        )
                    nc.vector.scalar_tensor_tensor(
                        y32[:, m, :].rearrange("p (b n) -> p b n", b=2),
                        po.rearrange("p (b n) -> p b n", b=2),
                        bo_sb[:, m : m + 1],
                        xfv[:, m],
                        OP.add, OP.add,
                    )
                    nc.scalar.activation(
                        ybf[:, m, :], y32[:, m, :], AF.Identity,
                        bias=zero_sb[:], scale=1.0,
                    )

                # ---- FFN fused: h1 chunk -> gelu -> accumulate into 6 psum ----
                _ptags = ["st0", "st1", "at0", "at1", "x0", "x1"]
                pouts = [
                    pp.tile([128, NPAIR], f32, name=f"pout{o}", tag=_ptags[o])
                    for o in range(KC)
                ]
                for mf in range(KF):
                    p1 = pp.tile([128, NPAIR], f32, name="p1", tag="mm", bufs=2)
                    for k in range(KC):
                        nc.tensor.matmul(
                            p1[:],
                            w1_sb[:, k, 128 * mf : 128 * (mf + 1)],
                            ybf[:, k, :],
                            start=(k == 0),
                            stop=(k == KC - 1),
                        )
                    h1c = small.tile([128, NPAIR], bf16, name="h1c", tag="h1c")
                    nc.scalar.activation(
                        h1c[:], p1[:], AF.Gelu, bias=b1_sb[:, mf : mf + 1], scale=1.0
                    )
                    for o in range(KC):
                        nc.tensor.matmul(
                            pouts[o][:],
                            w2_sb[:, mf, 128 * o : 128 * (o + 1)],
                            h1c[:],
                            start=(mf == 0),
                            stop=(mf == KF - 1),
                        )

                # ---- residual 2 + store ----
                o32 = xio.tile([128, 2 * KC, N], f32, name="o32", tag="xf")
                o32v = o32.rearrange("p (b k) n -> p k b n", b=2)
                for o in range(KC):
                    nc.vector.scalar_tensor_tensor(
                        o32v[:, o],
                        pouts[o].rearrange("p (b n) -> p b n", b=2),
                        b2_sb[:, o : o + 1],
                        y32[:, o, :].rearrange("p (b n) -> p b n", b=2),
                        OP.add, OP.add,
                    )
                nc.sync.dma_start(
                    out_d.ap()[b0 : b0 + 2].rearrange(
                        "b (ko p) n -> p (b ko) n", p=128
                    ),
                    o32[:],
                )

    nc.finalize()
    return nc


_CACHE = {}


def prepare_in_maps(inputs):
    x = np.asarray(inputs["x"], dtype=np.float32)  # (64, 768, 14, 14)
    ln_g = np.asarray(inputs["ln_g"], dtype=np.float32)
    ln_b = np.asarray(inputs["ln_b"], dtype=np.float32)
    wq = np.asarray(inputs["wq"], dtype=np.float32)
    bq = np.asarray(inputs["bq"], dtype=np.float32)
    wk = np.asarray(inputs["wk"], dtype=np.float32)
    bk = np.asarray(inputs["bk"], dtype=np.float32)
    wv = np.asarray(inputs["wv"], dtype=np.float32)
    bv = np.asarray(inputs["bv"], dtype=np.float32)
    wo = np.asarray(inputs["wo"], dtype=np.float32)
    bo = np.asarray(inputs["bo"], dtype=np.float32)
    w1 = np.asarray(inputs["w1"], dtype=np.float32)
    b1 = np.asarray(inputs["b1"], dtype=np.float32)
    w2 = np.asarray(inputs["w2"], dtype=np.float32)
    b2 = np.asarray(inputs["b2"], dtype=np.float32)
    rel_bias = np.asarray(inputs["rel_bias"], dtype=np.float32)

    bf = ml_dtypes.bfloat16

    # Fold LayerNorm gamma into QKV weights, beta into their biases.
    wqp_f = ln_g[:, None] * wq
    wkp_f = ln_g[:, None] * wk
    wvp_f = ln_g[:, None] * wv
    bqp = (ln_b @ wq + bq).astype(np.float32)
    bkp = (ln_b @ wk + bk).astype(np.float32)
    bvp = (ln_b @ wv + bv).astype(np.float32)
    # V bias commutes through softmax (rows sum to 1): fold into out-proj bias.
    bop = (bo + bvp @ wo).astype(np.float32)
    # Column sums of the folded Q/K weights for the post-hoc mean correction.
    # Use the bf16-rounded weights so the correction matches the matmul.
    wqp = wqp_f.astype(bf)
    wkp = wkp_f.astype(bf)
    wsq = wqp.astype(np.float32).sum(axis=0).astype(np.float32)
    wsk = wkp.astype(np.float32).sum(axis=0).astype(np.float32)

    # Relative position bias, transposed per head, exponentiated.
    rel_idx = _relative_indices()
    bias = rel_bias[:, rel_idx]  # (HEADS, N, N) : bias[h, n, m]
    ebT = np.exp(bias.transpose(0, 2, 1))  # (HEADS, m, n)
    eb_m = ebT.transpose(1, 0, 2)  # (m, HEADS, n)
    eb0 = np.ascontiguousarray(eb_m[:M0]).astype(bf)
    eb1 = np.ascontiguousarray(eb_m[M0:]).astype(bf)

    common = {
        "wq": wqp, "wk": wkp, "wv": wvp_f.astype(bf),
        "wo": wo.astype(bf), "w1": w1.astype(bf), "w2": w2.astype(bf),
        "bq": bqp, "bk": bkp, "bo": bop,
        "b1": b1.astype(np.float32), "b2": b2.astype(np.float32),
        "wsq": wsq, "wsk": wsk,
        "eb0": eb0, "eb1": eb1,
    }

    x_flat = x.reshape(B, C, N)
    in_maps = []
    for c in range(NCORES):
        m = dict(common)
        m["x"] = np.ascontiguousarray(x_flat[c * BPC : (c + 1) * BPC])
        in_maps.append(m)
    return in_maps


def kernel(**inputs):
    import sys

    if "/opt/trn_rl_repo" not in sys.path:
        sys.path.insert(0, "/opt/trn_rl_repo")
    from concourse.bass_utils import run_bass_kernel_spmd

    in_maps = prepare_in_maps(inputs)

    if "nc" not in _CACHE:
        _CACHE["nc"] = _build_bass()
    nc = _CACHE["nc"]

    res = run_bass_kernel_spmd(nc, in_maps, core_ids=list(range(NCORES)))
    _CACHE["last_res"] = res
    outs = [r["out"] for r in res.results]
    full = np.concatenate(outs, axis=0)  # (64, 768, 196)
    return full.reshape(B, C, H, W).astype(np.float32)



# revision 29
# speedup vs baseline: 1.1920x; 1.1920x over previous
"""CoAtNet transformer block kernel for Trainium2 (8 NeuronCores).

Strategy:
  - Data-parallel over batch: 64 images -> 8 per core, no collectives.
  - Channel-major activation layout [C, N] on chip; batches in pairs
    (moving free dim 392).
  - LayerNorm computed explicitly per pair, software-pipelined one pair
    ahead (stats matmuls + chain + xn run during the previous pair's
    attention/FFN), so Q/K/V all consume normalized xn directly.
  - fp8(e4m3) DoubleRow matmuls for the V projection, out projection and
    FFN1 (weights x64, activations x4 pre-scale); Q/K/FFN2 stay bf16 for
    accuracy (rel-err budget 2e-2).
  - Attention: k-major sim (simT[m,n]) per head; exp on scalar engine;
    exp(rel-bias) multiplier on gpsimd/vector; the AV matmul is packed
    column-tiled with a 64-wide ones stationary so the softmax
    denominator lands broadcast across partitions 64..127 of the same
    PSUM tile; normalization is then a single DVE divide per head
    (no 1-partition reciprocals, no partition broadcasts).
  - FFN: per 128-wide h1 chunk, gelu then accumulate into six persistent
    output PSUM banks (no full h1 materialization).
"""

import numpy as np
import ml_dtypes

H = 14
W = 14
C = 768
HEADS = 12
EXPAND = 4
N = H * W  # 196
B = 64
NCORES = 8
BPC = B // NCORES  # 8 batches per core
DH = C // HEADS  # 64
KC = C // 128  # 6 chunks of 128 channels
K2 = KC // 2  # 3 chunk-pairs for DoubleRow
F = C * EXPAND  # 3072
KF = F // 128  # 24
NPAIR = 2 * N  # 392
NP_PAD = 400  # fp8 pair-tile token stride (16B aligned)
M0, M1 = 128, N - 128  # token chunks 128 + 68
MCHUNKS = ((0, M0), (M0, M1))
SW = 64.0  # fp8 weight pre-scale
SX = 4.0  # fp8 activation pre-scale
SV = 16.0  # v_sb carries 16*v for fp8-range of OT


def _relative_indices():
    gy, gx = np.meshgrid(np.arange(H), np.arange(W), indexing="ij")
    py, px = gy.reshape(-1), gx.reshape(-1)
    rel_y = py[None, :] - py[:, None] + H
    rel_x = px[None, :] - px[:, None] + W
    return rel_y * W + rel_x  # (N, N) int


def _build_bass(dbg=None):
    import concourse.bacc as bacc
    import concourse.mybir as mybir
    import concourse.tile as tile

    f32 = mybir.dt.float32
    bf16 = mybir.dt.bfloat16
    fp8 = mybir.dt.float8e4
    AF = mybir.ActivationFunctionType
    OP = mybir.AluOpType
    DR = mybir.MatmulPerfMode.DoubleRow

    nc = bacc.Bacc("TRN2")

    # ---- DRAM parameters (per core) ----
    x_in = nc.declare_dram_parameter("x", [BPC, C, N], f32, isOutput=False)
    wq_d = nc.declare_dram_parameter("wq", [C, C], bf16, isOutput=False)
    wk_d = nc.declare_dram_parameter("wk", [C, C], bf16, isOutput=False)
    wv_d = nc.declare_dram_parameter("wv", [C, C], fp8, isOutput=False)
    wo_d = nc.declare_dram_parameter("wo", [C, C], fp8, isOutput=False)
    w1_d = nc.declare_dram_parameter("w1", [C, F], fp8, isOutput=False)
    w2_d = nc.declare_dram_parameter("w2", [F, C], bf16, isOutput=False)
    bq_d = nc.declare_dram_parameter("bq", [C], f32, isOutput=False)
    bk_d = nc.declare_dram_parameter("bk", [C], f32, isOutput=False)
    bo_d = nc.declare_dram_parameter("bo", [C], f32, isOutput=False)  # bo+bv@wo
    b1_d = nc.declare_dram_parameter("b1", [F], f32, isOutput=False)
    b2_d = nc.declare_dram_parameter("b2", [C], f32, isOutput=False)  # b2+bo_t
    # exp(biasT): [128, chunk, HEADS, N], m = chunk*128 + p
    eb_d = nc.declare_dram_parameter("eb", [128, 2, HEADS, N], bf16, isOutput=False)
    out_d = nc.declare_dram_parameter("out", [BPC, C, N], f32, isOutput=True)

    def ld(pool, name, dram, shape, pat, **axes):
        t = pool.tile(shape, dram.dtype, name=name)
        nc.sync.dma_start(
            t[:], dram.ap().rearrange(pat, **axes) if pat else dram.ap()
        )
        return t

    with tile.TileContext(nc) as tc:
        with (
            tc.tile_pool(name="wpool", bufs=1) as wpool,
            tc.tile_pool(name="acts", bufs=1) as acts,
            tc.tile_pool(name="prep", bufs=2) as prep,
            tc.tile_pool(name="xio", bufs=2) as xio,
            tc.tile_pool(name="small", bufs=2) as small,
            tc.tile_pool(name="psum", bufs=1, space="PSUM") as pp,
        ):
            def load_x(pair):
                b0 = 2 * pair
                t = xio.tile([128, 2 * KC, N], f32, name="xf", tag="xf")
                nc.sync.dma_start(
                    t[:],
                    x_in.ap()[b0 : b0 + 2].rearrange(
                        "b (ko p) n -> p (b ko) n", p=128
                    ),
                )
                return t

            # ---- resident weights; issue order = DMA priority order ----
            next_xf = load_x(0)
            wq_sb = ld(wpool, "wq_sb", wq_d, [128, KC, C], "(ko p) m -> p ko m", p=128)
            wk_sb = ld(wpool, "wk_sb", wk_d, [128, KC, C], "(ko p) m -> p ko m", p=128)
            bq_sb = ld(wpool, "bq_sb", bq_d, [128, KC], "(mo p) -> p mo", p=128)
            bk_sb = ld(wpool, "bk_sb", bk_d, [128, KC], "(mo p) -> p mo", p=128)
            wv_sb = ld(
                wpool, "wv_sb", wv_d, [128, K2, 2, C],
                "(k2 ki p) m -> p k2 ki m", p=128, ki=2,
            )
            eb_sb = ld(wpool, "eb_sb", eb_d, [128, 2, HEADS, N], None)
            wo_sb = ld(
                wpool, "wo_sb", wo_d, [128, K2, 2, C],
                "(k2 ki p) m -> p k2 ki m", p=128, ki=2,
            )
            bo_sb = ld(wpool, "bo_sb", bo_d, [128, KC], "(mo p) -> p mo", p=128)
            b1_sb = ld(wpool, "b1_sb", b1_d, [128, KF], "(mo p) -> p mo", p=128)
            b2_sb = ld(wpool, "b2_sb", b2_d, [128, KC], "(mo p) -> p mo", p=128)
            w1_sb = ld(
                wpool, "w1_sb", w1_d, [128, K2, 2, F],
                "(k2 ki p) m -> p k2 ki m", p=128, ki=2,
            )
            w2_sb = ld(wpool, "w2_sb", w2_d, [128, KF, C], "(ko p) m -> p ko m", p=128)

            ones8 = wpool.tile([128, 1], fp8, name="ones8")
            nc.vector.memset(ones8[:], 1.0)
            eps_sb = wpool.tile([1, 1], f32, name="eps_sb")
            nc.vector.memset(eps_sb[:], 1e-5)

            def prep_pair(xf):
                """LN stats + xn for one pair; returns (xn_bf, xn8)."""
                xfv = xf.rearrange("p (b k) n -> p k b n", b=2)
                x8 = prep.tile([128, KC, 2, N], fp8, name="x8", tag="x8")
                xsq8 = prep.tile([128, KC, 2, N], fp8, name="xsq8", tag="xsq8")
                st_ps = pp.tile([64, NPAIR], f32, name="st_ps", tag="sst", padded_shape=[64, 512])
                for k in range(KC):
                    nc.vector.tensor_copy(x8[:, k], xfv[:, k])
                    nc.gpsimd.tensor_mul(xsq8[:, k], x8[:, k], x8[:, k])
                # one PSUM accumulation group at a time per bank
                for k in range(KC):
                    nc.tensor.matmul(
                        st_ps[0:1, :], ones8[:], x8[:, k],
                        start=(k == 0), stop=(k == KC - 1),
                    )
                for k in range(KC):
                    nc.tensor.matmul(
                        st_ps[32:33, :], ones8[:], xsq8[:, k],
                        start=(k == 0), stop=(k == KC - 1),
                    )
                mu = small.tile([1, NPAIR], f32, name="mu", tag="mu")
                nc.vector.tensor_scalar_mul(mu[:], st_ps[0:1, :], 1.0 / C)
                var = small.tile([1, NPAIR], f32, name="var", tag="var")
                nc.vector.scalar_tensor_tensor(
                    var[:], mu[:], -1.0, mu[:], OP.mult, OP.mult
                )
                nc.vector.scalar_tensor_tensor(
                    var[:], st_ps[32:33, :], 1.0 / C, var[:], OP.mult, OP.add
                )
                std = small.tile([1, NPAIR], f32, name="std", tag="std")
                nc.scalar.activation(
                    std[:], var[:], AF.Sqrt, bias=eps_sb[:], scale=1.0
                )
                rstd = small.tile([1, NPAIR], f32, name="rstd", tag="rstd")
                nc.vector.reciprocal_approx_fast(rstd[:], std[:])
                mr = small.tile([1, NPAIR], f32, name="mr", tag="mr")
                nc.vector.scalar_tensor_tensor(
                    mr[:], mu[:], -1.0, rstd[:], OP.mult, OP.mult
                )
                rstd_b = prep.tile([128, NPAIR], f32, name="rstd_b", tag="rstd_b")
                nc.gpsimd.partition_broadcast(rstd_b[:], rstd[:])
                mr_b = prep.tile([128, NPAIR], f32, name="mr_b", tag="mr_b")
                nc.gpsimd.partition_broadcast(mr_b[:], mr[:])
                rstd_bv = rstd_b.rearrange("p (b n) -> p b n", b=2)
                mr_bv = mr_b.rearrange("p (b n) -> p b n", b=2)

                xn_bf = prep.tile([128, KC, NPAIR], bf16, name="xn_bf", tag="xn_bf")
                # token dim padded to 512 so V-proj DR stationaries can always
                # be 128 wide (tok+128 <= 512 for tok=324); pad zeroed.
                xn8 = prep.tile([128, K2, 2, 512], fp8, name="xn8", tag="xn8")
                nc.vector.memset(xn8[:, :, :, NPAIR:], 0.0)
                xnv = xn_bf.rearrange("p k (b n) -> p k b n", b=2)
                for k in range(KC):
                    t = small.tile([128, NPAIR], f32, name="t_ln", tag="t_ln")
                    tv = t.rearrange("p (b n) -> p b n", b=2)
                    nc.gpsimd.tensor_mul(tv, xfv[:, k], rstd_bv)
                    nc.vector.tensor_add(xnv[:, k], tv, mr_bv)
                    nc.vector.tensor_scalar_mul(
                        xn8[:, k // 2, k % 2, 0:NPAIR], xn_bf[:, k], SX
                    )
                return xn_bf, xn8

            pending = prep_pair(next_xf)

            for pair in range(BPC // 2):
                b0 = 2 * pair
                xf = next_xf
                xfv = xf.rearrange("p (b k) n -> p k b n", b=2)
                xn_bf, xn8 = pending
                if pair + 1 < BPC // 2:
                    next_xf = load_x(pair + 1)

                # ---- Q/K projections (bf16, from xn) ----
                qT = acts.tile([128, KC, NPAIR], bf16, name="qT", tag="qT")
                kT = acts.tile([128, KC, NPAIR], bf16, name="kT", tag="kT")
                for dst, w_sb, bias_sb in ((qT, wq_sb, bq_sb), (kT, wk_sb, bk_sb)):
                    for m in range(KC):
                        ps = pp.tile([128, NPAIR], f32, name="ps_qk", tag="mm", bufs=2, padded_shape=[128, 512])
                        for k in range(KC):
                            nc.tensor.matmul(
                                ps[:],
                                w_sb[:, k, 128 * m : 128 * (m + 1)],
                                xn_bf[:, k],
                                start=(k == 0),
                                stop=(k == KC - 1),
                            )
                        nc.vector.tensor_scalar_add(
                            dst[:, m], ps[:], bias_sb[:, m : m + 1]
                        )

                # ---- V projection (fp8 DoubleRow, token-major stationary) ----
                v_sb = []
                for j in range(2):
                    v_c = []
                    for ci, (mstart, mlen) in enumerate(MCHUNKS):
                        # cols 0:64 = v, cols 64:128 = ones (softmax denom
                        # rides the same matmul/accumulation group as AV)
                        vt = acts.tile(
                            [128, HEADS, 128], bf16, name=f"v_{j}_{ci}",
                            tag=f"v_{j}_{ci}",
                        )
                        nc.vector.memset(vt[:, :, DH:], 1.0)
                        pvs = [
                            pp.tile([128, 384], f32, name=f"pv{s}", tag="mm", bufs=2, padded_shape=[128, 512])
                            for s in range(2)
                        ]
                        tok = j * N + mstart
                        for k2 in range(K2):
                            for s in range(2):
                                nc.tensor.matmul(
                                    pvs[s][:],
                                    xn8[:, k2, :, tok : tok + 128],
                                    wv_sb[:, k2, :, 384 * s : 384 * (s + 1)],
                                    start=(k2 == 0),
                                    stop=(k2 == K2 - 1),
                                    perf_mode=DR,
                                )
                        for s in range(2):
                            nc.vector.tensor_scalar_mul(
                                vt[:mlen, 6 * s : 6 * (s + 1), 0:DH],
                                pvs[s][:mlen].rearrange("p (h d) -> p h d", h=6),
                                SV / (SX * SW),
                            )
                        v_c.append(vt)
                    v_sb.append(v_c)

                # ---- prep next pair (overlaps attention/FFN below) ----
                if pair + 1 < BPC // 2:
                    pending = prep_pair(next_xf)

                # ---- attention ----
                OT8 = acts.tile([128, K2, 2, NP_PAD], fp8, name="OT8", tag="OT8")
                dbg_att = None
                if dbg in ("den", "av", "sim0", "et0"):
                    dbg_att = acts.tile(
                        [128, KC, NPAIR], f32, name="dbg_att", tag="dbg_att"
                    )

                def emit_sims(j, hp):
                    cb = j * N
                    ets = []
                    for hh in range(2):
                        prow = 64 * hh
                        h = 2 * hp + hh
                        sim = pp.tile([128, 2, N], f32, name="sim", tag=f"st{hh}", padded_shape=[128, 2, 256])
                        for ci, (mstart, mlen) in enumerate(MCHUNKS):
                            nc.tensor.matmul(
                                sim[:mlen, ci],
                                kT[prow : prow + 64, hp,
                                   cb + mstart : cb + mstart + mlen],
                                qT[prow : prow + 64, hp, cb : cb + N],
                                start=True, stop=True,
                            )
                        et = small.tile([128, 2, N], bf16, name="et", tag=f"et{hh}")
                        for ci, (mstart, mlen) in enumerate(MCHUNKS):
                            nc.scalar.activation(
                                et[:mlen, ci], sim[:mlen, ci], AF.Exp,
                                bias=0.0, scale=1.0,
                            )
                            nc.vector.tensor_mul(
                                et[:mlen, ci], et[:mlen, ci],
                                eb_sb[:mlen, ci, h, :],
                            )
                        if dbg == "sim0" and j == 0 and hp == 0:
                            for ci in range(2):
                                nc.vector.tensor_copy(
                                    dbg_att[:, ci, 196 * hh : 196 * hh + 196],
                                    sim[:, ci, :],
                                )
                        if dbg == "et0" and j == 0 and hp == 0:
                            for ci in range(2):
                                nc.vector.tensor_copy(
                                    dbg_att[:, ci, 196 * hh : 196 * hh + 196],
                                    et[:, ci, :],
                                )
                        ets.append(et)
                    return ets

                def emit_omms(j, hp, ets):
                    cb = j * N
                    for hh in range(2):
                        h = 2 * hp + hh
                        ops = pp.tile([128, N], f32, name="ops", tag=f"at{hh}", padded_shape=[128, 512])
                        for ci, (mstart, mlen) in enumerate(MCHUNKS):
                            nc.tensor.matmul(
                                ops[:],
                                v_sb[j][ci][:mlen, h, :],
                                ets[hh][:mlen, ci],
                                start=(ci == 0), stop=(ci == 1),
                            )
                        den = small.tile([64, N], f32, name="den", tag=f"den{hh}")
                        nc.vector.tensor_copy(den[:], ops[64:128])
                        rec = small.tile([64, N], f32, name="rec", tag=f"rec{hh}")
                        nc.vector.reciprocal_approx_fast(rec[:], den[:])
                        nc.vector.tensor_mul(
                            OT8[64 * hh : 64 * hh + 64, hp // 2, hp % 2,
                                cb : cb + N],
                            ops[0:DH],
                            rec[:],
                        )
                        if dbg_att is not None:
                            nc.vector.tensor_copy(
                                dbg_att[64 * hh : 64 * hh + 64, hp, cb : cb + N],
                                ops[64:128] if dbg == "den" else ops[0:DH],
                            )

                prev = None
                for j in range(2):
                    for hp in range(HEADS // 2):
                        ets = emit_sims(j, hp)
                        if prev is not None:
                            emit_omms(*prev)
                        prev = (j, hp, ets)
                emit_omms(*prev)

                # ---- out projection (fp8 DR) + residual 1 ----
                y32 = acts.tile([128, KC, NPAIR], f32, name="y32", tag="y32")
                y8 = acts.tile([128, K2, 2, NP_PAD], fp8, name="y8", tag="y8")
                for m in range(KC):
                    po = pp.tile([128, NPAIR], f32, name="po", tag="mm", bufs=2, padded_shape=[128, 512])
                    for k2 in range(K2):
                        nc.tensor.matmul(
                            po[:],
                            wo_sb[:, k2, :, 128 * m : 128 * (m + 1)],
                            OT8[:, k2, :, 0:NPAIR],
                            start=(k2 == 0),
                            stop=(k2 == K2 - 1),
                            perf_mode=DR,
                        )
                    nc.vector.scalar_tensor_tensor(
                        y32[:, m, :].rearrange("p (b n) -> p b n", b=2),
                        po.rearrange("p (b n) -> p b n", b=2),
                        1.0 / (SV * SW),
                        xfv[:, m],
                        OP.mult, OP.add,
                    )
                    nc.vector.tensor_scalar(
                        y8[:, m // 2, m % 2, 0:NPAIR], y32[:, m, :],
                        bo_sb[:, m : m + 1], SX, OP.add, OP.mult,
                    )

                # ---- FFN: fp8 DR h1 chunks -> gelu(bf16) -> bf16 accumulate ----
                _ptags = ["st0", "st1", "at0", "at1", "sst", "p5"]
                pouts = [
                    pp.tile([128, NPAIR], f32, name=f"pout{o}", tag=_ptags[o], padded_shape=[128, 512])
                    for o in range(KC)
                ]
                for mf in range(KF):
                    p1 = pp.tile([128, NPAIR], f32, name="p1", tag="mm", bufs=2, padded_shape=[128, 512])
                    for k2 in range(K2):
                        nc.tensor.matmul(
                            p1[:],
                            w1_sb[:, k2, :, 128 * mf : 128 * (mf + 1)],
                            y8[:, k2, :, 0:NPAIR],
                            start=(k2 == 0),
                            stop=(k2 == K2 - 1),
                            perf_mode=DR,
                        )
                    h1c = small.tile([128, NPAIR], bf16, name="h1c", tag="h1c")
                    nc.scalar.activation(
                        h1c[:], p1[:], AF.Gelu, bias=b1_sb[:, mf : mf + 1],
                        scale=1.0 / (SX * SW),
                    )
                    for o in range(KC):
                        nc.tensor.matmul(
                            pouts[o][:],
                            w2_sb[:, mf, 128 * o : 128 * (o + 1)],
                            h1c[:],
                            start=(mf == 0),
                            stop=(mf == KF - 1),
                        )

                # ---- residual 2 + store ----
                o32 = xio.tile([128, 2 * KC, N], f32, name="o32", tag="xf")
                o32v = o32.rearrange("p (b k) n -> p k b n", b=2)
                for o in range(KC):
                    nc.vector.scalar_tensor_tensor(
                        o32v[:, o],
                        pouts[o].rearrange("p (b n) -> p b n", b=2),
                        b2_sb[:, o : o + 1],
                        y32[:, o, :].rearrange("p (b n) -> p b n", b=2),
                        OP.add, OP.add,
                    )
                if dbg == "v0":
                    nc.vector.tensor_copy(o32[:, 0:HEADS, 0:DH], v_sb[0][0][:])
                elif dbg in ("den", "av", "sim0", "et0"):
                    sv = dbg_att.rearrange("p k (b n) -> p k b n", b=2)
                    for k in range(KC):
                        nc.vector.tensor_copy(o32v[:, k], sv[:, k])
                elif dbg == "ot":
                    for k2 in range(K2):
                        for ki in range(2):
                            nc.vector.tensor_copy(
                                o32v[:, 2 * k2 + ki],
                                OT8[:, k2, ki, 0:NPAIR].rearrange(
                                    "p (b n) -> p b n", b=2
                                ),
                            )
                elif dbg is not None:
                    src = {"xn": xn_bf, "qt": qT, "kt": kT, "y32": y32}[dbg]
                    sv = src.rearrange("p k (b n) -> p k b n", b=2)
                    for k in range(KC):
                        nc.vector.tensor_copy(o32v[:, k], sv[:, k])
                nc.sync.dma_start(
                    out_d.ap()[b0 : b0 + 2].rearrange(
                        "b (ko p) n -> p (b ko) n", p=128
                    ),
                    o32[:],
                )

    nc.finalize()
    return nc


_CACHE = {}


def prepare_in_maps(inputs):
    x = np.asarray(inputs["x"], dtype=np.float32)  # (64, 768, 14, 14)
    ln_g = np.asarray(inputs["ln_g"], dtype=np.float32)
    ln_b = np.asarray(inputs["ln_b"], dtype=np.float32)
    wq = np.asarray(inputs["wq"], dtype=np.float32)
    bq = np.asarray(inputs["bq"], dtype=np.float32)
    wk = np.asarray(inputs["wk"], dtype=np.float32)
    bk = np.asarray(inputs["bk"], dtype=np.float32)
    wv = np.asarray(inputs["wv"], dtype=np.float32)
    bv = np.asarray(inputs["bv"], dtype=np.float32)
    wo = np.asarray(inputs["wo"], dtype=np.float32)
    bo = np.asarray(inputs["bo"], dtype=np.float32)
    w1 = np.asarray(inputs["w1"], dtype=np.float32)
    b1 = np.asarray(inputs["b1"], dtype=np.float32)
    w2 = np.asarray(inputs["w2"], dtype=np.float32)
    b2 = np.asarray(inputs["b2"], dtype=np.float32)
    rel_bias = np.asarray(inputs["rel_bias"], dtype=np.float32)

    bf = ml_dtypes.bfloat16
    e4 = ml_dtypes.float8_e4m3

    def q8(a):
        return np.clip(a * SW, -240.0, 240.0).astype(e4)

    # Fold LayerNorm gamma into QKV weights, beta into their biases.
    wqp = (ln_g[:, None] * wq).astype(bf)
    wkp = (ln_g[:, None] * wk).astype(bf)
    wvp8 = q8(ln_g[:, None] * wv)
    bqp = (ln_b @ wq + bq).astype(np.float32)
    bkp = (ln_b @ wk + bk).astype(np.float32)
    bvp = (ln_b @ wv + bv).astype(np.float32)
    # V bias commutes through softmax (rows sum to 1): fold into out-proj bias.
    bo_t = (bo + bvp @ wo).astype(np.float32)

    # Relative position bias, transposed per head, exponentiated,
    # laid out [p, chunk, HEADS, n] with m = chunk*128 + p.
    rel_idx = _relative_indices()
    bias = rel_bias[:, rel_idx]  # (HEADS, n, m)
    ebT = np.exp(bias.transpose(0, 2, 1))  # (HEADS, m, n)
    eb_m = ebT.transpose(1, 0, 2)  # (m, HEADS, n)
    eb_pad = np.zeros((256, HEADS, N), np.float32)
    eb_pad[:N] = eb_m
    eb_arr = np.ascontiguousarray(
        eb_pad.reshape(2, 128, HEADS, N).transpose(1, 0, 2, 3)
    ).astype(bf)

    common = {
        "wq": wqp, "wk": wkp, "wv": wvp8,
        "wo": q8(wo), "w1": q8(w1), "w2": w2.astype(bf),
        "bq": bqp, "bk": bkp,
        "bo": bo_t,
        "b1": b1.astype(np.float32),
        "b2": (b2 + bo_t).astype(np.float32),
        "eb": eb_arr,
    }

    x_flat = x.reshape(B, C, N)
    in_maps = []
    for c in range(NCORES):
        m = dict(common)
        m["x"] = np.ascontiguousarray(x_flat[c * BPC : (c + 1) * BPC])
        in_maps.append(m)
    return in_maps


def kernel(**inputs):
    import sys

    if "/opt/trn_rl_repo" not in sys.path:
        sys.path.insert(0, "/opt/trn_rl_repo")
    from concourse.bass_utils import run_bass_kernel_spmd

    in_maps = prepare_in_maps(inputs)

    import os

    dbg = os.environ.get("KERNEL_DBG") or None
    key = f"nc{dbg}"
    if key not in _CACHE:
        _CACHE[key] = _build_bass(dbg)
    nc = _CACHE[key]

    res = run_bass_kernel_spmd(nc, in_maps, core_ids=list(range(NCORES)))
    _CACHE["last_res"] = res
    outs = [r["out"] for r in res.results]
    full = np.concatenate(outs, axis=0)  # (64, 768, 196)
    return full.reshape(B, C, H, W).astype(np.float32)
